# revision 1
# baseline (speedup 1.0000x reference)
"""Cosine-similarity retrieval kernel for Trainium2 (8 NeuronCores, SPMD).

Computes out[q, n] = cos(query[q], support[n]) for query [2048, 512] and
support [50000, 512], out [2048, 50000] float32 — matching
torch.nn.CosineSimilarity semantics (dots / max(|q|*|s|, 1e-8)).

Strategy:
  * Shard support on the N axis: 8 shards of 6250 rows. Each core reads only
    its shard plus the (replicated, small) query set and writes its own
    [2048, 6250] column block of the output; the full output is assembled on
    the host — no device collective needed.
  * Rows are pre-normalized on the host (norms in float64), so the device
    kernel is a pure matmul Qn @ Sn^T; the PSUM result IS the cosine.
  * Both operands are staged transposed ([D, *]) so the contraction dim D
    lands on SBUF partitions; the matmul streams the support shard with the
    query tile as the stationary operand.
  * Matmul dtype is float32r (fp32 storage, ~12-bit mantissa in the PE) at
    full 1 cycle/row streaming rate; storage/DMA stays plain fp32.
"""

import os

import numpy as np

QN, DN, NN = 2048, 512, 50000
N_CORES = 8
NSH = NN // N_CORES  # 6250 support rows per core
P = 128
KT = DN // P  # 4 contraction chunks
QT = QN // P  # 16 query tiles
N_CHUNKS = 13  # per-core n tiling; 6250/13 keeps every matmul N >= 256
# qT is loaded in column chunks interleaved with the first sT loads so the
# first matmuls start as early as possible.
QCHUNKS = [(c, 512) for c in range(0, QN, 512)]
ST_PREFETCH = 4  # sT chunk double-buffer depth in the j-outer loop
QBATCH = 4  # q-tiles per merged output store
EPS = 1e-8

# "fp32r" (default): fp32 storage, float32r matmul (fast, ~2**-13 precision)
# "fp16": float16 storage+matmul. "fp32": exact fp32 matmul (4x slower PE).
DT_MODE = os.environ.get("COS_DT_MODE", "fp32r")
# Output staged as fp16 (halves the dominant HBM write traffic; host upcasts
# to f32; adds ~2.8e-4 L2 quantization). "fp32" restores exact staging.
OUT_MODE = os.environ.get("COS_OUT_DT", "fp16")

_PROGRAM = {}


def _chunks(total, n, granularity=1):
    # fp32r matmul requires an even moving free dim (ISA s3d3_mm_fp32r
    # restriction), so chunk at `granularity` then scale back up.
    assert total % granularity == 0
    units = total // granularity
    base, rem = divmod(units, n)
    sizes = [(base + 1) * granularity] * rem + [base * granularity] * (n - rem)
    out, start = [], 0
    for s in sizes:
        out.append((start, s))
        start += s
    return out


def _round_fp32r(x):
    """Round fp32 to the PE's float32r format: round-to-nearest-even keeping
    11 explicit mantissa bits (low 12 bits zeroed). Matches
    neuron_dtypes.fp32r.cast_fp32_to_fp32r for normal/zero values."""
    u = np.ascontiguousarray(x, dtype=np.float32).view(np.uint32)
    lsb = (u >> 12) & 1
    r = (u + np.uint32(0x7FF) + lsb) & np.uint32(0xFFFFF000)
    return r.view(np.float32)


def _patch_ldw_opt():
    """walrus's LDWEIGHTS dedup (--enable-ldw-opt) is hardcoded off in
    concourse; consecutive matmuls here share weights, so turn it on."""
    from concourse import bass_utils as bu

    if getattr(bu.run_command, "_ldw_patched", False):
        return
    orig = bu.run_command

    def patched(argv, **kwargs):
        if isinstance(argv, list) and "--enable-ldw-opt=false" in argv:
            argv = [
                "--enable-ldw-opt=true" if a == "--enable-ldw-opt=false" else a
                for a in argv
            ]
        return orig(argv, **kwargs)

    patched._ldw_patched = True
    bu.run_command = patched


def _build_program(dt_mode, out_mode):
    import concourse.bass as bass  # noqa: F401
    import concourse.tile as tile
    from concourse import bacc, mybir

    if os.environ.get("COS_LDW_OPT", "1") != "0":
        _patch_ldw_opt()

    if dt_mode == "fp16":
        store_dt = mybir.dt.float16
    elif dt_mode == "fp32":
        store_dt = mybir.dt.float32
    else:
        # float32r end-to-end: DMA moves bits, host pre-rounds, and the
        # walrus verifier sees properly-rounded fp32r feeding the matmul.
        store_dt = mybir.dt.float32r
    out_dt = mybir.dt.float16 if out_mode == "fp16" else mybir.dt.float32

    nc = bacc.Bacc(
        "TRN2", target_bir_lowering=False, debug=False, num_devices=N_CORES
    )
    qT = nc.dram_tensor("qT", [DN, QN], store_dt, kind="ExternalInput").ap()
    sT = nc.dram_tensor("sT", [DN, NSH], store_dt, kind="ExternalInput").ap()
    out = nc.dram_tensor("out", [QN, NSH], out_dt, kind="ExternalOutput").ap()

    chunks = _chunks(NSH, N_CHUNKS, granularity=2)
    max_nw = max(nw for _, nw in chunks)

    # 3D views putting the contraction (k) / q-tile (g) index on a middle
    # axis so one DMA instruction moves all 4 k-slices of a chunk (or all
    # QBATCH q-tiles of a store) — each dma_start costs ~0.6us of Sync
    # issue time, so instruction count matters.
    qT3 = qT.rearrange("(k p) q -> p k q", p=P)  # [128, KT, QN]
    sT3 = sT.rearrange("(k p) n -> p k n", p=P)  # [128, KT, NSH]
    out3 = out.rearrange("(g p) n -> p g n", p=P)  # [128, QT, NSH]

    with tile.TileContext(nc) as tc:
        with (
            tc.tile_pool(name="qw", bufs=1) as qpool,
            tc.tile_pool(name="sw", bufs=1) as spool,
            tc.tile_pool(name="ps", bufs=8, space="PSUM") as pspool,
            tc.tile_pool(name="ostage", bufs=4) as opool,
        ):
            qts = {}  # qchunk_idx -> resident [P, KT, 512] tile
            sts = {}  # j -> cycling [P, KT, max_nw] tile

            def load_qchunk(ci):
                c0, cw = QCHUNKS[ci]
                t = qpool.tile([P, KT, cw], store_dt, name=f"qTs{ci}", tag=f"qTs{ci}")
                nc.sync.dma_start(t[:], qT3[:, :, c0 : c0 + cw])
                qts[ci] = t

            def load_schunk(j):
                n0, nw = chunks[j]
                t = spool.tile(
                    [P, KT, max_nw],
                    store_dt,
                    name=f"sTs{j}",
                    tag="sTs",
                    bufs=ST_PREFETCH + 2,
                )
                nc.sync.dma_start(t[:, :, :nw], sT3[:, :, n0 : n0 + nw])
                sts[j] = t

            # First loads interleaved per-k so the k=0 matmul of (q0, j0)
            # unblocks after just two ~256KB DMAs; then qT column chunks
            # interleave with the next sT chunks so the j=0 pass (which
            # sweeps all q-tiles) isn't weight-starved.
            qc0_0, qc0_w = QCHUNKS[0]
            n0_0, nw_0 = chunks[0]
            tq0 = qpool.tile([P, KT, qc0_w], store_dt, name="qTs0", tag="qTs0")
            ts0 = spool.tile(
                [P, KT, max_nw], store_dt, name="sTs0", tag="sTs", bufs=ST_PREFETCH + 2
            )
            for k in range(KT):
                nc.sync.dma_start(tq0[:, k, :], qT3[:, k, qc0_0 : qc0_0 + qc0_w])
                nc.sync.dma_start(ts0[:, k, :nw_0], sT3[:, k, n0_0 : n0_0 + nw_0])
            qts[0] = tq0
            sts[0] = ts0
            qc_next = 1
            for j in range(1, ST_PREFETCH):
                load_schunk(j)
                if qc_next < len(QCHUNKS):
                    load_qchunk(qc_next)
                    qc_next += 1
            while qc_next < len(QCHUNKS):
                load_qchunk(qc_next)
                qc_next += 1

            def q_weight(k, qi):
                ci, off = divmod(qi * P, 512)
                return qts[ci][:, k, off : off + P]

            copy_idx = 0
            # j outer / q inner: each j-pass reuses one ~1MB sT chunk for
            # all 16 q-tiles, so the DMA feed never starves the PE (the
            # q-outer order would need the whole shard per pass).
            for j, (n0, nw) in enumerate(chunks):
                if j + ST_PREFETCH < N_CHUNKS:
                    load_schunk(j + ST_PREFETCH)
                # Final j-pass stores per q-tile so the last store (on the
                # critical path into the kernel-exit barrier) is 4x smaller.
                qbatch = 1 if j == N_CHUNKS - 1 else QBATCH
                for qg in range(QT // qbatch):
                    ot = opool.tile([P, QBATCH, max_nw], out_dt, name="ot", tag="ot")
                    for qb in range(qbatch):
                        qi = qg * qbatch + qb
                        ps = pspool.tile(
                            [P, 512], mybir.dt.float32, name="ps", tag="ps"
                        )
                        for k in range(KT):
                            nc.tensor.matmul(
                                ps[:, :nw],
                                lhsT=q_weight(k, qi),
                                rhs=sts[j][:, k, :nw],
                                start=(k == 0),
                                stop=(k == KT - 1),
                            )
                        # split PSUM->SBUF copies (with downcast) ACT/DVE
                        if copy_idx % 2 == 0:
                            nc.scalar.copy(out=ot[:, qb, :nw], in_=ps[:, :nw])
                        else:
                            nc.vector.tensor_copy(out=ot[:, qb, :nw], in_=ps[:, :nw])
                        copy_idx += 1
                    nc.sync.dma_start(
                        out3[:, qg * qbatch : qg * qbatch + qbatch, n0 : n0 + nw],
                        ot[:, :qbatch, :nw],
                    )
    nc.compile()
    return nc


def _get_program(dt_mode=None, out_mode=None):
    key = (dt_mode or DT_MODE, out_mode or OUT_MODE)
    if key not in _PROGRAM:
        _PROGRAM[key] = _build_program(*key)
    return _PROGRAM[key]


def _prep_inputs(support_set, query_set, dt_mode=None):
    dt_mode = dt_mode or DT_MODE
    S = np.asarray(support_set, dtype=np.float32)
    Q = np.asarray(query_set, dtype=np.float32)
    assert S.shape == (NN, DN) and Q.shape == (QN, DN)

    host_dt = np.float16 if dt_mode == "fp16" else np.float32

    def normalize(x):
        x64 = x.astype(np.float64)
        norm = np.sqrt(np.einsum("nd,nd->n", x64, x64))
        # Reference divides by max(|q|*|s|, eps). Norms here are ~22, so the
        # eps clamp never binds for real rows; an all-zero row would give
        # dots == 0 in the reference too, so map inv-norm to 0 there.
        inv = np.where(norm > 0, 1.0 / np.maximum(norm, EPS), 0.0)
        return (x64 * inv[:, None]).astype(host_dt)

    Sn = normalize(S)
    Qn = normalize(Q)
    if dt_mode == "fp32r":
        Sn = _round_fp32r(Sn)
        Qn = _round_fp32r(Qn)
    qT = np.ascontiguousarray(Qn.T)  # [512, 2048]
    in_maps = []
    for c in range(N_CORES):
        sT = np.ascontiguousarray(Sn[c * NSH : (c + 1) * NSH].T)  # [512, 6250]
        in_maps.append({"qT": qT, "sT": sT})
    return in_maps


def _run(in_maps, dt_mode=None, out_mode=None, trace=False, **kwargs):
    from concourse import bass_utils

    nc = _get_program(dt_mode, out_mode)
    return bass_utils.run_bass_kernel_spmd(
        nc, in_maps, core_ids=list(range(N_CORES)), trace=trace, **kwargs
    )


def _assemble(results):
    return np.concatenate(
        [np.asarray(results[c]["out"], dtype=np.float32) for c in range(N_CORES)],
        axis=1,
    )


def kernel(support_set, query_set):
    in_maps = _prep_inputs(support_set, query_set)
    res = _run(in_maps)
    return _assemble(res.results)



# revision 6
# speedup vs baseline: 1.1401x; 1.1401x over previous
"""Cosine-similarity retrieval kernel for Trainium2 (8 NeuronCores, SPMD).

Computes out[q, n] = cos(query[q], support[n]) for query [2048, 512] and
support [50000, 512], out [2048, 50000] float32 — matching
torch.nn.CosineSimilarity semantics (dots / max(|q|*|s|, 1e-8)).

Strategy:
  * Shard support on the N axis: 8 shards of 6250 rows (zero-padded to 6272 =
    49 blocks of 128). Each core reads its shard plus the replicated query
    set and writes its own [6272, 2048] output block (n-major, i.e. the
    transpose of the final layout); the host trims/transposes/concatenates —
    no device collective needed.
  * Rows are pre-normalized on the host (norms in float64), so the device
    kernel is a pure matmul; the PSUM result IS the cosine.
  * Storage/matmul dtype is fp16 (1 cycle/row on the PE, same as fp32r, but
    weights go through the LDWEIGHTS+FWL path instead of per-matmul fp32
    self-loading). The support block [128d, 128n] is the STATIONARY operand,
    reused across 4 consecutive matmuls that stream the resident query set
    512 columns at a time; with walrus --enable-ldw-opt the LDWEIGHTS for
    repeats is deduped, so weight-load overhead amortizes 4x and prefetches
    into the PE background buffer during the preceding matmuls.
  * PSUM: 4 banks accumulate one n-block over the 4 k-slices (bank = [128,
    512] fp32 = exactly one 2KB bank); the other 4 banks drain the previous
    n-block through ACT/DVE fp32->fp16 copies, so the PE never waits.
  * Output staged fp16 (halves the dominant HBM write traffic; host upcasts;
    ~2.4e-4 extra rel-L2). One store per n-block: 4KB-contiguous per
    partition, and the final store is only 0.5MB so the kernel-exit barrier
    isn't stuck behind a big trailing DMA.
"""

import os

import numpy as np

QN, DN, NN = 2048, 512, 50000
N_CORES = 8
NSH = NN // N_CORES  # 6250 support rows per core
P = 128
KT = DN // P  # 4 contraction slices
NBLK = (NSH + P - 1) // P  # 49 n-blocks per core
NSHP = NBLK * P  # 6272 (22 zero-padded rows, trimmed on host)
QC = 4  # query chunks, each one PSUM bank wide
QW = QN // QC  # 512 fp32 = one full PSUM bank
# n-blocks per DMA slab: small first slab so the first matmul unblocks after
# ~0.3MB of DMA; 1MB slabs after that for 2KB-contiguous packets.
SLAB_BLOCKS = [2, 8, 8, 8, 8, 8, 7]
SLAB_PREFETCH = 3
EPS = 1e-8

# "fp16" (default), "bf16", or "fp32r": SBUF/DMA storage + matmul dtype.
DT_MODE = os.environ.get("COS_DT_MODE", "fp16")
# Output staging dtype: "fp16" (default) or "fp32".
OUT_MODE = os.environ.get("COS_OUT_DT", "fp16")

_PROGRAM = {}


def _patch_ldw_opt():
    """walrus's LDWEIGHTS dedup (--enable-ldw-opt) is hardcoded off in
    concourse; consecutive matmuls here share weights, so turn it on."""
    from concourse import bass_utils as bu

    if getattr(bu.run_command, "_ldw_patched", False):
        return
    orig = bu.run_command

    def patched(argv, **kwargs):
        if isinstance(argv, list) and "--enable-ldw-opt=false" in argv:
            argv = [
                "--enable-ldw-opt=true" if a == "--enable-ldw-opt=false" else a
                for a in argv
            ]
        return orig(argv, **kwargs)

    patched._ldw_patched = True
    bu.run_command = patched


def _build_program(dt_mode, out_mode):
    import concourse.bass as bass  # noqa: F401
    import concourse.tile as tile
    from concourse import bacc, mybir

    if os.environ.get("COS_LDW_OPT", "1") != "0":
        _patch_ldw_opt()

    store_dt = {
        "fp16": mybir.dt.float16,
        "bf16": mybir.dt.bfloat16,
        "fp32r": mybir.dt.float32r,
    }[dt_mode]
    out_dt = mybir.dt.float16 if out_mode == "fp16" else mybir.dt.float32

    nc = bacc.Bacc(
        "TRN2", target_bir_lowering=False, debug=False, num_devices=N_CORES
    )
    qT = nc.dram_tensor("qT", [DN, QN], store_dt, kind="ExternalInput").ap()
    sT = nc.dram_tensor("sT", [DN, NSHP], store_dt, kind="ExternalInput").ap()
    out = nc.dram_tensor("out", [NSHP, QN], out_dt, kind="ExternalOutput").ap()

    qT3 = qT.rearrange("(k p) q -> p k q", p=P)  # [128, KT, QN]
    sT3 = sT.rearrange("(k p) n -> p k n", p=P)  # [128, KT, NSHP]
    out3 = out.rearrange("(s p) q -> p s q", p=P)  # [128, NBLK, QN]

    slab_off, o = [], 0
    for nb in SLAB_BLOCKS:
        slab_off.append(o)
        o += nb
    assert o == NBLK

    with tile.TileContext(nc) as tc:
        with (
            tc.tile_pool(name="qw", bufs=1) as qpool,
            tc.tile_pool(name="sw", bufs=SLAB_PREFETCH + 1) as spool,
            tc.tile_pool(name="ps", bufs=8, space="PSUM") as pspool,
            tc.tile_pool(name="ostage", bufs=4) as opool,
        ):
            qt = qpool.tile([P, KT, QN], store_dt, name="qt", tag="qt")
            slabs = {}

            def load_slab(si, with_q=False):
                nb = SLAB_BLOCKS[si]
                n0 = slab_off[si] * P
                w = nb * P
                t = spool.tile(
                    [P, KT, 8 * P],
                    store_dt,
                    name=f"s{si}",
                    tag="ss",
                    bufs=SLAB_PREFETCH + 1,
                )
                if with_q:
                    # k-interleaved with the query loads so the (k=0, qc=0)
                    # matmul unblocks after just two small DMAs.
                    for k in range(KT):
                        nc.sync.dma_start(t[:, k, :w], sT3[:, k, n0 : n0 + w])
                        nc.sync.dma_start(qt[:, k, :], qT3[:, k, :])
                else:
                    nc.sync.dma_start(t[:, :, :w], sT3[:, :, n0 : n0 + w])
                slabs[si] = t

            load_slab(0, with_q=True)
            for si in range(1, SLAB_PREFETCH):
                load_slab(si)

            copy_idx = 0
            for si, nb in enumerate(SLAB_BLOCKS):
                if si + SLAB_PREFETCH < len(SLAB_BLOCKS):
                    load_slab(si + SLAB_PREFETCH)
                for b in range(nb):
                    sb = slab_off[si] + b
                    pss = [
                        pspool.tile(
                            [P, QW], mybir.dt.float32, name="ps", tag="ps"
                        )
                        for _ in range(QC)
                    ]
                    # k outer / qc inner: the 4 qc matmuls share one
                    # stationary [128, 128] support block -> 1 LDWEIGHTS per
                    # (block, k) after walrus dedup, prefetched during the
                    # previous k's matmuls.
                    for k in range(KT):
                        wt = slabs[si][:, k, b * P : (b + 1) * P]
                        for qc in range(QC):
                            nc.tensor.matmul(
                                pss[qc][:, :],
                                lhsT=wt,
                                rhs=qt[:, k, qc * QW : (qc + 1) * QW],
                                start=(k == 0),
                                stop=(k == KT - 1),
                            )
                    ot = opool.tile([P, QN], out_dt, name="ot", tag="ot")
                    # split PSUM->SBUF copies (with downcast) across ACT/DVE
                    for qc in range(QC):
                        dst = ot[:, qc * QW : (qc + 1) * QW]
                        if copy_idx % 2 == 0:
                            nc.scalar.copy(out=dst, in_=pss[qc][:, :])
                        else:
                            nc.vector.tensor_copy(out=dst, in_=pss[qc][:, :])
                        copy_idx += 1
                    nc.sync.dma_start(out3[:, sb, :], ot[:, :])
    nc.compile()
    return nc


def _get_program(dt_mode=None, out_mode=None):
    key = (dt_mode or DT_MODE, out_mode or OUT_MODE)
    if key not in _PROGRAM:
        _PROGRAM[key] = _build_program(*key)
    return _PROGRAM[key]


def _round_fp32r(x):
    """Round fp32 to the PE's float32r format: round-to-nearest-even keeping
    11 explicit mantissa bits (low 12 bits zeroed)."""
    u = np.ascontiguousarray(x, dtype=np.float32).view(np.uint32)
    lsb = (u >> 12) & 1
    r = (u + np.uint32(0x7FF) + lsb) & np.uint32(0xFFFFF000)
    return r.view(np.float32)


def _host_dt(dt_mode):
    if dt_mode == "fp16":
        return np.float16
    if dt_mode == "fp32r":
        return np.float32
    from ml_dtypes import bfloat16

    return bfloat16


def _prep_inputs(support_set, query_set, dt_mode=None):
    dt_mode = dt_mode or DT_MODE
    S = np.asarray(support_set, dtype=np.float32)
    Q = np.asarray(query_set, dtype=np.float32)
    assert S.shape == (NN, DN) and Q.shape == (QN, DN)
    hdt = _host_dt(dt_mode)

    def normalize(x):
        x64 = x.astype(np.float64)
        norm = np.sqrt(np.einsum("nd,nd->n", x64, x64))
        # Reference divides by max(|q|*|s|, eps). Norms here are ~22, so the
        # eps clamp never binds for real rows; an all-zero row would give
        # dots == 0 in the reference too, so map inv-norm to 0 there.
        inv = np.where(norm > 0, 1.0 / np.maximum(norm, EPS), 0.0)
        return x64 * inv[:, None]

    Sn = normalize(S)
    Qn = normalize(Q)
    qT = np.ascontiguousarray(Qn.T).astype(hdt)  # [512, 2048]
    if dt_mode == "fp32r":
        qT = _round_fp32r(qT)
    in_maps = []
    for c in range(N_CORES):
        sT = np.zeros((DN, NSHP), dtype=hdt)
        sT[:, :NSH] = np.ascontiguousarray(Sn[c * NSH : (c + 1) * NSH].T).astype(
            hdt
        )
        if dt_mode == "fp32r":
            sT = _round_fp32r(sT)
        in_maps.append({"qT": qT, "sT": sT})
    return in_maps


def _run(in_maps, dt_mode=None, out_mode=None, trace=False, **kwargs):
    from concourse import bass_utils

    nc = _get_program(dt_mode, out_mode)
    return bass_utils.run_bass_kernel_spmd(
        nc, in_maps, core_ids=list(range(N_CORES)), trace=trace, **kwargs
    )


def _assemble(results):
    out = np.empty((QN, NN), dtype=np.float32)
    for c in range(N_CORES):
        blk = np.asarray(results[c]["out"])[:NSH]  # [6250, 2048]
        out[:, c * NSH : (c + 1) * NSH] = blk.T
    return out


def kernel(support_set, query_set):
    in_maps = _prep_inputs(support_set, query_set)
    res = _run(in_maps)
    return _assemble(res.results)
